# revision 5
# baseline (speedup 1.0000x reference)
"""Bass/Trainium2 kernel for nn_DotProductAttention (B=32, Q=K=1024, D=512).

Strategy: valid_lens-aware work-balanced sharding. K-tiles beyond a
batch's valid_len produce exp(-1e6)=0 and contribute nothing, so the
scores-stage and out-stage matmuls for them are skipped. The schedule
is built at kernel-call time (valid_lens are an input):

  - Each batch is split into two 512-query "halves"; the 64 halves are
    sorted by K-tile count kt=ceil(vl/128) and snake-assigned to 8
    slots x 8 cores. Slot s has a compile-time tile count
    KU_s = max kt within its rank group, identical on every core, so
    one SPMD program serves all cores with zero cycle imbalance.
  - Each unit (core, slot) computes UNNORMALIZED partial softmax
    outputs ov = sum_k exp*V, ol = sum_k exp*L plus the denominator
    row; the host divides (and could sum k-split partials). This
    removes the on-device denominator transpose round-trip,
    reciprocal, and output scaling.
  - All inputs are pre-laid on host in exact SBUF image layout, so
    every DMA is a plain contiguous [128, X] copy.

Per unit, everything stays transposed (feature/key dim on the SBUF
partition axis) so no on-device transposes are needed:

  projT[e,q]   = W^T.T-tiles @ qT          (contract d)
  scoresT[k,q] = kT-tiles.T  @ projT       (contract e)
  expT[k,q]    = exp(scoresT/sqrt(d) + maskbias[k])  (masked -> 0)
  den[q]       = ones.T @ sum_kt expT      (DVE partial sums + 1 matmul)
  ov[q,v]      = expT-slices.T @ values    (unnormalized)

The softmax max-subtraction is dropped: scores/sqrt(d) ~ N(0,1) here,
so exp() cannot overflow. All matmuls run in bf16 with fp32 PSUM.
"""

import numpy as np
import ml_dtypes

import concourse.bass as bass
import concourse.mybir as mybir
from concourse import tile
from concourse.bacc import Bacc
from concourse.bass_utils import run_bass_kernel_spmd

BF16 = mybir.dt.bfloat16
F32 = mybir.dt.float32
AF = mybir.ActivationFunctionType
bf = ml_dtypes.bfloat16

B, Q, K, D = 32, 1024, 1024, 512
N_CORES = 8
SCALE = 1.0 / float(np.sqrt(D))
MASK_VALUE = -1000000.0

ET, DT = D // 128, D // 128       # 4 feature tiles of 128
QU = 512                          # query columns per unit
QT_U = QU // 128                  # 4 q-tiles per unit
S = 8                             # slots (units) per core


def plan_units(valid_lens):
    """Snake-assign sorted (batch, q-half) pairs to 8 slots x 8 cores.

    Returns list of (KU_s, grp) per slot, grp[c] = (g, h, ktg) or None.
    Slots are in descending-KU order; KU_s == 0 slots are dead.
    """
    vl = np.asarray(valid_lens).astype(np.int64)
    kt = np.ceil(vl / 128.0).astype(int)
    halves = []
    for g in range(B):
        if kt[g] > 0:
            halves.append((g, 0, int(kt[g])))
            halves.append((g, 1, int(kt[g])))
    halves.sort(key=lambda x: -x[2])
    while len(halves) < S * N_CORES:
        halves.append(None)
    slots = []
    for s in range(S):
        grp = halves[N_CORES * s : N_CORES * (s + 1)]
        ku = max((x[2] for x in grp if x is not None), default=0)
        slots.append((ku, grp))
    return slots


def build_program(kus) -> bass.Bass:
    """kus: per-slot K-tile counts (descending, same on all cores)."""
    nc = Bacc()

    active = [(s, ku) for s, ku in enumerate(kus) if ku > 0]
    sa = len(active)

    wt_d = nc.dram_tensor("wt", (128, DT * D), BF16, kind="ExternalInput")
    q_d = nc.dram_tensor("q", (sa, 128, DT * QU), BF16, kind="ExternalInput")
    kt_ds, v_ds, l_ds, mb_ds = [], [], [], []
    for si, (s, ku) in enumerate(active):
        kt_ds.append(
            nc.dram_tensor(f"kt{s}", (128, ET * ku * 128), BF16, kind="ExternalInput")
        )
        v_ds.append(
            nc.dram_tensor(f"v{s}", (128, ku * D), BF16, kind="ExternalInput")
        )
        l_ds.append(
            nc.dram_tensor(f"l{s}", (128, ku * D), BF16, kind="ExternalInput")
        )
        mb_ds.append(nc.dram_tensor(f"mb{s}", (128, ku), F32, kind="ExternalInput"))
    ov_d = nc.dram_tensor("ov", (sa, 128, QT_U * D), BF16, kind="ExternalOutput")
    ol_d = nc.dram_tensor("ol", (sa, 128, QT_U * D), BF16, kind="ExternalOutput")
    den_d = nc.dram_tensor("den", (sa, QU), F32, kind="ExternalOutput")

    with tile.TileContext(nc) as tc:
        with (
            tc.tile_pool(name="wpool", bufs=1) as wpool,
            tc.tile_pool(name="inpool", bufs=2) as inpool,
            tc.tile_pool(name="workpool", bufs=2) as workpool,
            tc.tile_pool(name="outpool", bufs=2) as outpool,
            tc.tile_pool(name="ps_acc", bufs=4, space="PSUM") as ps_acc,
            tc.tile_pool(name="ps_out", bufs=4, space="PSUM") as ps_out,
        ):
            wt_sb = wpool.tile([128, DT, D], BF16, tag="wt")
            nc.sync.dma_start(wt_sb[:], wt_d[:])
            ones_sb = wpool.tile([128, 1], F32, tag="ones")
            nc.vector.memset(ones_sb[:], 1.0)

            # warm the PE HAM clock-gate during the initial input DMAs:
            # ~4us of dummy matmuls flips the clock 1.2 -> 2.4 GHz before
            # the first real matmul issues
            warm_sb = wpool.tile([128, 512], BF16, tag="warm")
            nc.vector.memset(warm_sb[:], 0.0)
            ps_warm = ps_acc.tile([128, 512], F32, tag="ps_acc")
            for _ in range(10):
                nc.tensor.matmul(
                    ps_warm[:], warm_sb[:, 0:128], warm_sb[:], start=True, stop=True
                )

            for si, (s, ku) in enumerate(active):
                q_sb = inpool.tile([128, DT, QU], BF16, tag="q")
                kt_sb = inpool.tile([128, ET, ku * 128], BF16, tag="kt")
                v_sb = inpool.tile([128, ku, D], BF16, tag="v")
                l_sb = inpool.tile([128, ku, D], BF16, tag="l")
                nc.sync.dma_start(q_sb[:], q_d[si])
                nc.sync.dma_start(kt_sb[:], kt_ds[si][:])
                # bounce maskbias onto the ACT engine so downstream exp
                # activations wait on same-engine program order, not a DMA sem
                mb_raw = workpool.tile([128, ku], F32, tag="mb_raw")
                mb_sb = workpool.tile([128, ku], F32, tag="mb")
                nc.sync.dma_start(mb_raw[:], mb_ds[si][:])
                nc.scalar.copy(mb_sb[:], mb_raw[:])
                nc.sync.dma_start(v_sb[:], v_ds[si][:])
                nc.sync.dma_start(l_sb[:], l_ds[si][:])

                # ---- projT[e,q] = (q @ W.T).T, tiled ----
                proj_sb = workpool.tile([128, ET, QU], BF16, tag="proj")
                for et in range(ET):
                    ps = ps_acc.tile([128, QU], F32, tag="ps_acc")
                    for dt in range(DT):
                        nc.tensor.matmul(
                            ps[:],
                            wt_sb[:, dt, et * 128 : (et + 1) * 128],
                            q_sb[:, dt, :],
                            start=(dt == 0),
                            stop=(dt == DT - 1),
                        )
                    nc.scalar.copy(proj_sb[:, et, :], ps[:])

                # ---- scoresT[k,q] -> expT = exp(scores*SCALE + maskbias) ----
                exp_sb = workpool.tile([128, ku, QU], BF16, tag="exp")
                dacc = workpool.tile([128, QU], F32, tag="dacc")
                for kt in range(ku):
                    ps = ps_acc.tile([128, QU], F32, tag="ps_acc")
                    for et in range(ET):
                        nc.tensor.matmul(
                            ps[:],
                            kt_sb[:, et, kt * 128 : (kt + 1) * 128],
                            proj_sb[:, et, :],
                            start=(et == 0),
                            stop=(et == ET - 1),
                        )
                    nc.scalar.activation(
                        exp_sb[:, kt, :],
                        ps[:],
                        AF.Exp,
                        bias=mb_sb[:, kt : kt + 1],
                        scale=SCALE,
                    )
                    if kt == 1:
                        nc.vector.tensor_add(
                            dacc[:], exp_sb[:, 0, :], exp_sb[:, 1, :]
                        )
                    elif kt >= 2:
                        nc.vector.tensor_add(dacc[:], dacc[:], exp_sb[:, kt, :])

                # ---- den[q] = ones.T @ sum_kt expT ----
                if ku == 1:
                    nc.vector.tensor_copy(dacc[:], exp_sb[:, 0, :])
                psd = ps_acc.tile([1, QU], F32, tag="ps_acc")
                nc.tensor.matmul(psd[:], ones_sb[:], dacc[:], start=True, stop=True)
                denrow = workpool.tile([1, QU], F32, tag="denrow")
                nc.scalar.copy(denrow[:], psd[:])
                nc.sync.dma_start(den_d[si, :], denrow[0:1, :])

                # ---- unnormalized out: ov[q,v] = expT.T @ V, ol = expT.T @ L
                # psv/psl matmuls interleaved per kt (same stationary expT
                # slice back-to-back keeps the PE weight-load pipeline warm)
                ov_stage = outpool.tile([128, QT_U, D], BF16, tag="ov_stage")
                ol_stage = outpool.tile([128, QT_U, D], BF16, tag="ol_stage")
                for qt in range(QT_U):
                    psv = ps_out.tile([128, 512], F32, tag="ps_out")
                    psl = ps_out.tile([128, 512], F32, tag="ps_out")
                    for kt in range(ku):
                        lhs = exp_sb[:, kt, qt * 128 : (qt + 1) * 128]
                        nc.tensor.matmul(
                            psv[:], lhs, v_sb[:, kt, :],
                            start=(kt == 0), stop=(kt == ku - 1),
                        )
                        nc.tensor.matmul(
                            psl[:], lhs, l_sb[:, kt, :],
                            start=(kt == 0), stop=(kt == ku - 1),
                        )
                    # drain copies split across DVE (psv) and ACT (psl);
                    # per-qt output DMA keeps the post-last-matmul tail short
                    nc.vector.tensor_copy(ov_stage[:, qt, :], psv[:])
                    nc.scalar.copy(ol_stage[:, qt, :], psl[:])
                    nc.sync.dma_start(
                        ov_d[si][:, qt * D : (qt + 1) * D], ov_stage[:, qt, :]
                    )
                    nc.sync.dma_start(
                        ol_d[si][:, qt * D : (qt + 1) * D], ol_stage[:, qt, :]
                    )

    nc.finalize()
    # NOTE: an LDWEIGHTS-dedup pass (reuse stationary operand across paired
    # matmuls) was tried on the predecessor kernel and produced wrong
    # results on HW with zero time gain. Do not re-add.
    return nc


def make_in_maps(queries, keys, values, labels, W, valid_lens, slots):
    """Host-side shard + SBUF-image layout prep. All numpy, fp32->bf16."""
    q32 = np.asarray(queries, np.float32)
    k32 = np.asarray(keys, np.float32)
    v32 = np.asarray(values, np.float32)
    l32 = np.asarray(labels, np.float32)
    w32 = np.asarray(W, np.float32)
    vl = np.asarray(valid_lens).astype(np.int64)

    # wt[p, dt*D + e] = W[e, dt*128 + p]  (= W.T laid out d-tile-major)
    wt = np.ascontiguousarray(
        w32.T.reshape(DT, 128, D).transpose(1, 0, 2).reshape(128, DT * D)
    ).astype(bf)

    qT = q32.transpose(0, 2, 1).astype(bf)    # (B, D, Q)
    kT = k32.transpose(0, 2, 1).astype(bf)    # (B, D, K)
    v16 = v32.astype(bf)
    l16 = l32.astype(bf)
    # maskbias[b, k] = 0 if k < vl[b] else MASK_VALUE
    mb = np.where(
        np.arange(K)[None, :] < vl[:, None], 0.0, MASK_VALUE
    ).astype(np.float32)

    active = [(s, ku, grp) for s, (ku, grp) in enumerate(slots) if ku > 0]
    in_maps = []
    for c in range(N_CORES):
        m = {"wt": wt}
        q_all = np.zeros((len(active), 128, DT * QU), bf)
        for si, (s, ku, grp) in enumerate(active):
            kt_a = np.zeros((128, ET, ku * 128), bf)
            v_a = np.zeros((128, ku, D), bf)
            l_a = np.zeros((128, ku, D), bf)
            mb_a = np.full((128, ku), MASK_VALUE, np.float32)
            a = grp[c]
            if a is not None:
                g, h, ktg = a
                # q image: [p, dt, j] = qT[g][dt*128+p, h*QU+j]
                q_all[si] = (
                    qT[g][:, h * QU : (h + 1) * QU]
                    .reshape(DT, 128, QU)
                    .transpose(1, 0, 2)
                    .reshape(128, DT * QU)
                )
                kk = ktg * 128
                kt_a[:, :, :kk] = (
                    kT[g][:, :kk].reshape(ET, 128, kk).transpose(1, 0, 2)
                )
                v_a[:, :ktg, :] = (
                    v16[g][:kk].reshape(ktg, 128, D).transpose(1, 0, 2)
                )
                l_a[:, :ktg, :] = (
                    l16[g][:kk].reshape(ktg, 128, D).transpose(1, 0, 2)
                )
                mb_a[:, :ktg] = mb[g][:kk].reshape(ktg, 128).T
            m[f"kt{s}"] = np.ascontiguousarray(kt_a.reshape(128, ET * ku * 128))
            m[f"v{s}"] = np.ascontiguousarray(v_a.reshape(128, ku * D))
            m[f"l{s}"] = np.ascontiguousarray(l_a.reshape(128, ku * D))
            m[f"mb{s}"] = np.ascontiguousarray(mb_a)
        m["q"] = q_all
        in_maps.append(m)
    return in_maps


def _fixup_all_masked(out_v, out_l, values, labels, valid_lens):
    """valid_len==0 -> reference softmax is uniform over ALL positions."""
    vl = np.asarray(valid_lens).astype(np.int64)
    for b in np.nonzero(vl == 0)[0]:
        out_v[b, :, :] = np.asarray(values[b], np.float32).mean(axis=0)[None, :]
        out_l[b, :, :] = np.asarray(labels[b], np.float32).mean(axis=0)[None, :]
    return out_v, out_l


def run(queries, keys, values, labels, W, valid_lens, trace=False):
    slots = plan_units(valid_lens)
    kus = [ku for ku, _ in slots]
    out_v = np.zeros((B, Q, D), np.float32)
    out_l = np.zeros((B, Q, D), np.float32)
    if not any(ku > 0 for ku in kus):  # all batches fully masked
        out_v, out_l = _fixup_all_masked(
            out_v, out_l, values, labels, valid_lens
        )
        return (out_v, out_l), None

    nc = build_program(kus)
    in_maps = make_in_maps(queries, keys, values, labels, W, valid_lens, slots)
    res = run_bass_kernel_spmd(nc, in_maps, list(range(N_CORES)), trace=trace)

    den = np.zeros((B, Q), np.float32)
    active = [(s, ku, grp) for s, (ku, grp) in enumerate(slots) if ku > 0]
    for c in range(N_CORES):
        rc = res.results[c]
        for si, (s, ku, grp) in enumerate(active):
            a = grp[c]
            if a is None:
                continue
            g, h, _ = a
            sl = slice(h * QU, (h + 1) * QU)
            out_v[g, sl, :] += (
                rc["ov"][si].astype(np.float32)
                .reshape(128, QT_U, D).transpose(1, 0, 2).reshape(QU, D)
            )
            out_l[g, sl, :] += (
                rc["ol"][si].astype(np.float32)
                .reshape(128, QT_U, D).transpose(1, 0, 2).reshape(QU, D)
            )
            den[g, sl] += rc["den"][si]
    nz = den > 0
    np.divide(out_v, den[:, :, None], out=out_v, where=nz[:, :, None])
    np.divide(out_l, den[:, :, None], out=out_l, where=nz[:, :, None])
    out_v, out_l = _fixup_all_masked(out_v, out_l, values, labels, valid_lens)
    return (out_v, out_l), res


def kernel(queries, keys, values, labels, W, valid_lens):
    (out_v, out_l), _ = run(queries, keys, values, labels, W, valid_lens, trace=False)
    return (out_v, out_l)


# revision 7
# speedup vs baseline: 1.0429x; 1.0429x over previous
"""Bass/Trainium2 kernel for nn_DotProductAttention (B=32, Q=K=1024, D=512).

Strategy: valid_lens-aware work-balanced sharding. K-tiles beyond a
batch's valid_len produce exp(-1e6)=0 and contribute nothing, so the
scores-stage and out-stage matmuls for them are skipped. The schedule
is built at kernel-call time (valid_lens are an input):

  - Each batch is split into two 512-query "halves"; the 64 halves are
    sorted by K-tile count kt=ceil(vl/128) and snake-assigned to 8
    slots x 8 cores. Slot s has a compile-time tile count
    KU_s = max kt within its rank group, identical on every core, so
    one SPMD program serves all cores with zero cycle imbalance.
  - Each unit (core, slot) computes UNNORMALIZED partial softmax
    outputs ov = sum_k exp*V, ol = sum_k exp*L plus the denominator
    row; the host divides (and could sum k-split partials). This
    removes the on-device denominator transpose round-trip,
    reciprocal, and output scaling.
  - All inputs are pre-laid on host in exact SBUF image layout, so
    every DMA is a plain contiguous [128, X] copy.

Per unit, everything stays transposed (feature/key dim on the SBUF
partition axis) so no on-device transposes are needed:

  projT[e,q]   = W^T.T-tiles @ qT          (contract d)
  scoresT[k,q] = kT-tiles.T  @ projT       (contract e)
  expT[k,q]    = exp(scoresT/sqrt(d) + maskbias[k])  (masked -> 0)
  den[q]       = ones.T @ sum_kt expT      (DVE partial sums + 1 matmul)
  ov[q,v]      = expT-slices.T @ values    (unnormalized)

The softmax max-subtraction is dropped: scores/sqrt(d) ~ N(0,1) here,
so exp() cannot overflow. All matmuls run in bf16 with fp32 PSUM.
"""

import numpy as np
import ml_dtypes

import concourse.bass as bass
import concourse.mybir as mybir
from concourse import tile
from concourse.bacc import Bacc
from concourse.bass_utils import run_bass_kernel_spmd

BF16 = mybir.dt.bfloat16
F32 = mybir.dt.float32
AF = mybir.ActivationFunctionType
bf = ml_dtypes.bfloat16

B, Q, K, D = 32, 1024, 1024, 512
N_CORES = 8
SCALE = 1.0 / float(np.sqrt(D))
MASK_VALUE = -1000000.0

ET, DT = D // 128, D // 128       # 4 feature tiles of 128
QU = 512                          # query columns per unit
QT_U = QU // 128                  # 4 q-tiles per unit
S = 8                             # slots (units) per core


def plan_units(valid_lens):
    """Snake-assign sorted (batch, q-half) pairs to 8 slots x 8 cores.

    Returns list of (KU_s, grp) per slot, grp[c] = (g, h, ktg) or None.
    Slots are in descending-KU order; KU_s == 0 slots are dead.
    """
    vl = np.asarray(valid_lens).astype(np.int64)
    kt = np.ceil(vl / 128.0).astype(int)
    halves = []
    for g in range(B):
        if kt[g] > 0:
            halves.append((g, 0, int(kt[g])))
            halves.append((g, 1, int(kt[g])))
    halves.sort(key=lambda x: -x[2])
    while len(halves) < S * N_CORES:
        halves.append(None)
    slots = []
    for s in range(S):
        grp = halves[N_CORES * s : N_CORES * (s + 1)]
        ku = max((x[2] for x in grp if x is not None), default=0)
        slots.append((ku, grp))
    return slots


def build_program(kus) -> bass.Bass:
    """kus: per-slot K-tile counts (descending, same on all cores)."""
    nc = Bacc()

    active = [(s, ku) for s, ku in enumerate(kus) if ku > 0]
    sa = len(active)

    wt_d = nc.dram_tensor("wt", (128, DT * D), BF16, kind="ExternalInput")
    q_d = nc.dram_tensor("q", (sa, 128, DT * QU), BF16, kind="ExternalInput")
    kt_ds, v_ds, l_ds, mb_ds = [], [], [], []
    for si, (s, ku) in enumerate(active):
        kt_ds.append(
            nc.dram_tensor(f"kt{s}", (128, ET * ku * 128), BF16, kind="ExternalInput")
        )
        v_ds.append(
            nc.dram_tensor(f"v{s}", (128, ku * D), BF16, kind="ExternalInput")
        )
        l_ds.append(
            nc.dram_tensor(f"l{s}", (128, ku * D), BF16, kind="ExternalInput")
        )
        mb_ds.append(nc.dram_tensor(f"mb{s}", (128, ku), F32, kind="ExternalInput"))
    ov_d = nc.dram_tensor("ov", (sa, 128, QT_U * D), BF16, kind="ExternalOutput")
    ol_d = nc.dram_tensor("ol", (sa, 128, QT_U * D), BF16, kind="ExternalOutput")
    den_d = nc.dram_tensor("den", (sa, QU), F32, kind="ExternalOutput")

    with tile.TileContext(nc) as tc:
        with (
            tc.tile_pool(name="wpool", bufs=1) as wpool,
            tc.tile_pool(name="inpool", bufs=2) as inpool,
            tc.tile_pool(name="workpool", bufs=2) as workpool,
            tc.tile_pool(name="outpool", bufs=2) as outpool,
            tc.tile_pool(name="ps_acc", bufs=4, space="PSUM") as ps_acc,
            tc.tile_pool(name="ps_out", bufs=4, space="PSUM") as ps_out,
        ):
            wt_sb = wpool.tile([128, DT, D], BF16, tag="wt")
            nc.sync.dma_start(wt_sb[:], wt_d[:])
            ones_sb = wpool.tile([128, 1], F32, tag="ones")
            nc.vector.memset(ones_sb[:], 1.0)

            # warm the PE HAM clock-gate during the initial input DMAs.
            # The NEFF preamble takes ~7.5us and the first real matmul can't
            # start before ~15us (wt+q DMA behind the preamble); ~8 cold
            # matmuls (427ns each) flip the clock 1.2 -> 2.4 GHz, and the
            # rest run warm so the bridge costs nothing but keeps the PE
            # busy so it can't re-throttle before the real stream begins.
            warm_sb = wpool.tile([128, 512], BF16, tag="warm")
            nc.vector.memset(warm_sb[:], 0.0)
            ps_warm = ps_acc.tile([128, 512], F32, tag="ps_acc")
            for _ in range(22):
                nc.tensor.matmul(
                    ps_warm[:], warm_sb[:, 0:128], warm_sb[:], start=True, stop=True
                )

            for si, (s, ku) in enumerate(active):
                q_sb = inpool.tile([128, DT, QU], BF16, tag="q")
                kt_sb = inpool.tile([128, ET, ku * 128], BF16, tag="kt")
                v_sb = inpool.tile([128, ku, D], BF16, tag="v")
                l_sb = inpool.tile([128, ku, D], BF16, tag="l")
                nc.sync.dma_start(q_sb[:], q_d[si])
                nc.sync.dma_start(kt_sb[:], kt_ds[si][:])
                # bounce maskbias onto the ACT engine so downstream exp
                # activations wait on same-engine program order, not a DMA sem
                mb_raw = workpool.tile([128, ku], F32, tag="mb_raw")
                mb_sb = workpool.tile([128, ku], F32, tag="mb")
                nc.sync.dma_start(mb_raw[:], mb_ds[si][:])
                nc.scalar.copy(mb_sb[:], mb_raw[:])
                nc.sync.dma_start(v_sb[:], v_ds[si][:])
                nc.sync.dma_start(l_sb[:], l_ds[si][:])

                # ---- projT[e,q] = (q @ W.T).T, tiled ----
                proj_sb = workpool.tile([128, ET, QU], BF16, tag="proj")
                for et in range(ET):
                    ps = ps_acc.tile([128, QU], F32, tag="ps_acc")
                    for dt in range(DT):
                        nc.tensor.matmul(
                            ps[:],
                            wt_sb[:, dt, et * 128 : (et + 1) * 128],
                            q_sb[:, dt, :],
                            start=(dt == 0),
                            stop=(dt == DT - 1),
                        )
                    nc.scalar.copy(proj_sb[:, et, :], ps[:])

                # ---- scoresT[k,q] -> expT = exp(scores*SCALE + maskbias) ----
                exp_sb = workpool.tile([128, ku, QU], BF16, tag="exp")
                dacc = workpool.tile([128, QU], F32, tag="dacc")
                for kt in range(ku):
                    ps = ps_acc.tile([128, QU], F32, tag="ps_acc")
                    for et in range(ET):
                        nc.tensor.matmul(
                            ps[:],
                            kt_sb[:, et, kt * 128 : (kt + 1) * 128],
                            proj_sb[:, et, :],
                            start=(et == 0),
                            stop=(et == ET - 1),
                        )
                    nc.scalar.activation(
                        exp_sb[:, kt, :],
                        ps[:],
                        AF.Exp,
                        bias=mb_sb[:, kt : kt + 1],
                        scale=SCALE,
                    )
                    if kt == 1:
                        nc.vector.tensor_add(
                            dacc[:], exp_sb[:, 0, :], exp_sb[:, 1, :]
                        )
                    elif kt >= 2:
                        nc.vector.tensor_add(dacc[:], dacc[:], exp_sb[:, kt, :])

                # ---- den[q] = ones.T @ sum_kt expT ----
                if ku == 1:
                    nc.vector.tensor_copy(dacc[:], exp_sb[:, 0, :])
                psd = ps_acc.tile([1, QU], F32, tag="ps_acc")
                nc.tensor.matmul(psd[:], ones_sb[:], dacc[:], start=True, stop=True)
                denrow = workpool.tile([1, QU], F32, tag="denrow")
                nc.scalar.copy(denrow[:], psd[:])
                nc.sync.dma_start(den_d[si, :], denrow[0:1, :])

                # ---- unnormalized out: ov[q,v] = expT.T @ V, ol = expT.T @ L
                # psv/psl matmuls interleaved per kt (same stationary expT
                # slice back-to-back keeps the PE weight-load pipeline warm)
                ov_stage = outpool.tile([128, QT_U, D], BF16, tag="ov_stage")
                ol_stage = outpool.tile([128, QT_U, D], BF16, tag="ol_stage")
                for qt in range(QT_U):
                    psv = ps_out.tile([128, 512], F32, tag="ps_out")
                    psl = ps_out.tile([128, 512], F32, tag="ps_out")
                    for kt in range(ku):
                        lhs = exp_sb[:, kt, qt * 128 : (qt + 1) * 128]
                        nc.tensor.matmul(
                            psv[:], lhs, v_sb[:, kt, :],
                            start=(kt == 0), stop=(kt == ku - 1),
                        )
                        nc.tensor.matmul(
                            psl[:], lhs, l_sb[:, kt, :],
                            start=(kt == 0), stop=(kt == ku - 1),
                        )
                    # drain copies split across DVE (psv) and ACT (psl);
                    # output DMA in halves so the post-last-matmul tail is
                    # one 0.25 MB transfer, with the first half overlapped
                    nc.vector.tensor_copy(ov_stage[:, qt, :], psv[:])
                    nc.scalar.copy(ol_stage[:, qt, :], psl[:])
                    if qt in (QT_U // 2 - 1, QT_U - 1):
                        h = 0 if qt < QT_U // 2 else QT_U // 2
                        sl = slice(h * D, (h + QT_U // 2) * D)
                        nc.sync.dma_start(
                            ov_d[si][:, sl], ov_stage[:, h : h + QT_U // 2, :]
                        )
                        nc.sync.dma_start(
                            ol_d[si][:, sl], ol_stage[:, h : h + QT_U // 2, :]
                        )

    nc.finalize()
    # NOTE: an LDWEIGHTS-dedup pass (reuse stationary operand across paired
    # matmuls) was tried on the predecessor kernel and produced wrong
    # results on HW with zero time gain. Do not re-add.
    return nc


def make_in_maps(queries, keys, values, labels, W, valid_lens, slots):
    """Host-side shard + SBUF-image layout prep. All numpy, fp32->bf16."""
    q32 = np.asarray(queries, np.float32)
    k32 = np.asarray(keys, np.float32)
    v32 = np.asarray(values, np.float32)
    l32 = np.asarray(labels, np.float32)
    w32 = np.asarray(W, np.float32)
    vl = np.asarray(valid_lens).astype(np.int64)

    # wt[p, dt*D + e] = W[e, dt*128 + p]  (= W.T laid out d-tile-major)
    wt = np.ascontiguousarray(
        w32.T.reshape(DT, 128, D).transpose(1, 0, 2).reshape(128, DT * D)
    ).astype(bf)

    qT = q32.transpose(0, 2, 1).astype(bf)    # (B, D, Q)
    kT = k32.transpose(0, 2, 1).astype(bf)    # (B, D, K)
    v16 = v32.astype(bf)
    l16 = l32.astype(bf)
    # maskbias[b, k] = 0 if k < vl[b] else MASK_VALUE
    mb = np.where(
        np.arange(K)[None, :] < vl[:, None], 0.0, MASK_VALUE
    ).astype(np.float32)

    active = [(s, ku, grp) for s, (ku, grp) in enumerate(slots) if ku > 0]
    in_maps = []
    for c in range(N_CORES):
        m = {"wt": wt}
        q_all = np.zeros((len(active), 128, DT * QU), bf)
        for si, (s, ku, grp) in enumerate(active):
            kt_a = np.zeros((128, ET, ku * 128), bf)
            v_a = np.zeros((128, ku, D), bf)
            l_a = np.zeros((128, ku, D), bf)
            mb_a = np.full((128, ku), MASK_VALUE, np.float32)
            a = grp[c]
            if a is not None:
                g, h, ktg = a
                # q image: [p, dt, j] = qT[g][dt*128+p, h*QU+j]
                q_all[si] = (
                    qT[g][:, h * QU : (h + 1) * QU]
                    .reshape(DT, 128, QU)
                    .transpose(1, 0, 2)
                    .reshape(128, DT * QU)
                )
                kk = ktg * 128
                kt_a[:, :, :kk] = (
                    kT[g][:, :kk].reshape(ET, 128, kk).transpose(1, 0, 2)
                )
                v_a[:, :ktg, :] = (
                    v16[g][:kk].reshape(ktg, 128, D).transpose(1, 0, 2)
                )
                l_a[:, :ktg, :] = (
                    l16[g][:kk].reshape(ktg, 128, D).transpose(1, 0, 2)
                )
                mb_a[:, :ktg] = mb[g][:kk].reshape(ktg, 128).T
            m[f"kt{s}"] = np.ascontiguousarray(kt_a.reshape(128, ET * ku * 128))
            m[f"v{s}"] = np.ascontiguousarray(v_a.reshape(128, ku * D))
            m[f"l{s}"] = np.ascontiguousarray(l_a.reshape(128, ku * D))
            m[f"mb{s}"] = np.ascontiguousarray(mb_a)
        m["q"] = q_all
        in_maps.append(m)
    return in_maps


def _fixup_all_masked(out_v, out_l, values, labels, valid_lens):
    """valid_len==0 -> reference softmax is uniform over ALL positions."""
    vl = np.asarray(valid_lens).astype(np.int64)
    for b in np.nonzero(vl == 0)[0]:
        out_v[b, :, :] = np.asarray(values[b], np.float32).mean(axis=0)[None, :]
        out_l[b, :, :] = np.asarray(labels[b], np.float32).mean(axis=0)[None, :]
    return out_v, out_l


def run(queries, keys, values, labels, W, valid_lens, trace=False):
    slots = plan_units(valid_lens)
    kus = [ku for ku, _ in slots]
    out_v = np.zeros((B, Q, D), np.float32)
    out_l = np.zeros((B, Q, D), np.float32)
    if not any(ku > 0 for ku in kus):  # all batches fully masked
        out_v, out_l = _fixup_all_masked(
            out_v, out_l, values, labels, valid_lens
        )
        return (out_v, out_l), None

    nc = build_program(kus)
    in_maps = make_in_maps(queries, keys, values, labels, W, valid_lens, slots)
    res = run_bass_kernel_spmd(nc, in_maps, list(range(N_CORES)), trace=trace)

    den = np.zeros((B, Q), np.float32)
    active = [(s, ku, grp) for s, (ku, grp) in enumerate(slots) if ku > 0]
    for c in range(N_CORES):
        rc = res.results[c]
        for si, (s, ku, grp) in enumerate(active):
            a = grp[c]
            if a is None:
                continue
            g, h, _ = a
            sl = slice(h * QU, (h + 1) * QU)
            out_v[g, sl, :] += (
                rc["ov"][si].astype(np.float32)
                .reshape(128, QT_U, D).transpose(1, 0, 2).reshape(QU, D)
            )
            out_l[g, sl, :] += (
                rc["ol"][si].astype(np.float32)
                .reshape(128, QT_U, D).transpose(1, 0, 2).reshape(QU, D)
            )
            den[g, sl] += rc["den"][si]
    nz = den > 0
    np.divide(out_v, den[:, :, None], out=out_v, where=nz[:, :, None])
    np.divide(out_l, den[:, :, None], out=out_l, where=nz[:, :, None])
    out_v, out_l = _fixup_all_masked(out_v, out_l, values, labels, valid_lens)
    return (out_v, out_l), res


def kernel(queries, keys, values, labels, W, valid_lens):
    (out_v, out_l), _ = run(queries, keys, values, labels, W, valid_lens, trace=False)
    return (out_v, out_l)
